# revision 1
# baseline (speedup 1.0000x reference)
"""Trainium2 Bass kernel for CombinedEmbedding.

reference: out[b,s,f] = W @ x[b,s,f] + pos_emb[s] + fmap_emb[f],
with x a one-hot [B,S,F,V] float32 tensor.

Strategy (8 NeuronCores, data-parallel over tokens):
  - flatten x to [16384 tokens, V=16384]; core c takes the contiguous
    2048-token slice (b = c//2, s in [32*(c%2), 32*(c%2)+32)).
  - per 128-token tile: one custom-DVE affine_mul_reduce
    (sum of x * iota == the one-hot index, exactly) recovers the token
    id; a per-tile indirect DMA gathers the matching 2KB rows of
    W^T [V, E]; two DVE adds apply fmap_emb[f] and pos_emb[s].
  - x tiles stream as full 8 MB rows, ping-ponged across the two HWDGE
    rings (sync / scalar) to hide inter-DMA gaps. iota is generated
    on-device by GpSimd. W^T, fmap rows and a per-core pos-row table
    are replicated inputs.
"""

import numpy as np

B, S, F, V, E = 4, 64, 64, 16384, 512
NCORES = 8
TOKENS = B * S * F            # 16384
TPC = TOKENS // NCORES        # 2048 tokens per core
P = 128                       # partitions
NTILES = TPC // P             # 16 token tiles per core
GROUP = 4                     # token tiles per gather/output group

_cache = {}


def _build():
    import concourse.bass as bass
    import concourse.bacc as bacc
    import concourse.mybir as mybir
    import concourse.tile as tile
    from concourse.alu_op_type import AluOpType

    nc = bacc.Bacc(trn_type="TRN2")
    x = nc.declare_dram_parameter("x", [TPC, V], mybir.dt.float32, isOutput=False)
    wt = nc.declare_dram_parameter("wt", [V, E], mybir.dt.float32, isOutput=False)
    pos2 = nc.declare_dram_parameter("pos2", [TPC, E], mybir.dt.float32, isOutput=False)
    fmap = nc.declare_dram_parameter("fmap", [F, E], mybir.dt.float32, isOutput=False)
    out = nc.declare_dram_parameter("out", [TPC, E], mybir.dt.float32, isOutput=True)

    # views
    x_t = x.rearrange("(t p) v -> t p v", p=P)               # [16,128,V]
    pos2_g = pos2.rearrange("(g tt p) e -> g p tt e", p=P, tt=GROUP)
    out_g = out.rearrange("(g tt p) e -> g p tt e", p=P, tt=GROUP)
    wt_flat = wt[:, :]

    rings = [nc.sync, nc.scalar]  # the two HWDGE rings

    VH = V // 2
    with tile.TileContext(nc) as tc:
        with (
            tc.tile_pool(name="px", bufs=3) as px,
            tc.tile_pool(name="pconst", bufs=1) as pconst,
            tc.tile_pool(name="pscr", bufs=2) as pscr,
            tc.tile_pool(name="pidx", bufs=1) as pidx,
            tc.tile_pool(name="pg", bufs=2) as pg,
        ):
            iota_sb = pconst.tile([P, V], mybir.dt.int16)
            for h in range(2):
                nc.gpsimd.iota(
                    iota_sb[:, h * VH:(h + 1) * VH],
                    pattern=[[1, VH]], base=h * VH, channel_multiplier=0,
                )

            fmap_sb = pconst.tile([P, E], mybir.dt.float32)
            nc.gpsimd.dma_start(out=fmap_sb[0:F, :], in_=fmap[:, :])
            nc.gpsimd.dma_start(out=fmap_sb[F:P, :], in_=fmap[:, :])

            idx_all = pidx.tile([P, NTILES], mybir.dt.float32)
            dummy = pidx.tile([P, 1], mybir.dt.float32)

            for g in range(NTILES // GROUP):
                for tt in range(GROUP):
                    t = g * GROUP + tt
                    idx_tmp = pscr.tile([P, 2], mybir.dt.float32, tag="idx_tmp")
                    for h in range(2):
                        xt = px.tile([P, VH], mybir.dt.float32, tag="x")
                        rings[(2 * t + h) % 2].dma_start(
                            out=xt[:, :], in_=x_t[t, :, h * VH:(h + 1) * VH]
                        )
                        # one-hot: sum(x * iota) over the half == idx or 0.
                        nc.vector.affine_mul_reduce(
                            out=dummy.broadcast_to((P, VH)),
                            accum_out=idx_tmp[:, h:h + 1],
                            in0=xt[:, :],
                            in1=iota_sb[:, h * VH:(h + 1) * VH],
                            scale=1.0,
                            bias=0.0,
                        )
                    nc.vector.tensor_add(
                        out=idx_all[:, t:t + 1],
                        in0=idx_tmp[:, 0:1],
                        in1=idx_tmp[:, 1:2],
                    )

                # gather W^T rows for this group's tokens
                idx_i = pscr.tile([P, GROUP], mybir.dt.int32, tag="idx_i")
                nc.vector.tensor_copy(
                    idx_i[:, :], idx_all[:, g * GROUP:(g + 1) * GROUP]
                )
                gath = pg.tile([P, GROUP, E], mybir.dt.float32, tag="gath")
                for tt in range(GROUP):
                    nc.gpsimd.indirect_dma_start(
                        out=gath[:, tt, :],
                        out_offset=None,
                        in_=wt_flat,
                        in_offset=bass.IndirectOffsetOnAxis(
                            ap=idx_i[:, tt:tt + 1], axis=0
                        ),
                    )
                posg = pg.tile([P, GROUP, E], mybir.dt.float32, tag="pos")
                nc.gpsimd.dma_start(out=posg[:, :, :], in_=pos2_g[g])
                outg = pg.tile([P, GROUP, E], mybir.dt.float32, tag="out")
                for tt in range(GROUP):
                    nc.vector.tensor_tensor(
                        out=gath[:, tt, :],
                        in0=gath[:, tt, :],
                        in1=fmap_sb[:, :],
                        op=AluOpType.add,
                    )
                    nc.vector.tensor_tensor(
                        out=outg[:, tt, :],
                        in0=gath[:, tt, :],
                        in1=posg[:, tt, :],
                        op=AluOpType.add,
                    )
                nc.gpsimd.dma_start(out=out_g[g], in_=outg[:, :, :])
    nc.finalize()
    return nc


def _host_shards(x, W, pos_emb, fmap_emb):
    x_flat = np.ascontiguousarray(x.reshape(TOKENS, V))
    wt = np.ascontiguousarray(W.T)                      # [V, E]
    fmap = np.ascontiguousarray(fmap_emb[:F])           # [64, E]
    in_maps = []
    for c in range(NCORES):
        s_base = (c % (S // 32)) * 32
        s_rows = pos_emb[s_base:s_base + TPC // F]      # [32, E]
        pos2 = np.repeat(s_rows, F, axis=0)             # [2048, E]
        in_maps.append({
            "x": x_flat[c * TPC:(c + 1) * TPC],
            "wt": wt,
            "pos2": np.ascontiguousarray(pos2),
            "fmap": fmap,
        })
    return in_maps


def kernel(x, W, pos_emb, fmap_emb):
    from concourse import bass_utils

    x = np.asarray(x, dtype=np.float32)
    W = np.asarray(W, dtype=np.float32)
    pos_emb = np.asarray(pos_emb, dtype=np.float32)
    fmap_emb = np.asarray(fmap_emb, dtype=np.float32)

    if "nc" not in _cache:
        _cache["nc"] = _build()
    nc = _cache["nc"]

    in_maps = _host_shards(x, W, pos_emb, fmap_emb)
    res = bass_utils.run_bass_kernel_spmd(nc, in_maps, core_ids=list(range(NCORES)))
    outs = [res.results[c]["out"] for c in range(NCORES)]
    full = np.concatenate(outs, axis=0).reshape(B, S, F, E)
    return full



# revision 5
# speedup vs baseline: 2.6229x; 2.6229x over previous
"""Trainium2 Bass kernel for CombinedEmbedding.

reference: out[b,s,f] = W @ x[b,s,f] + pos_emb[s] + fmap_emb[f],
with x a one-hot [B,S,F,V] float32 tensor.

Strategy (8 NeuronCores, data-parallel over tokens):
  - x's fp32 one-hot is transported as the high byte of each float
    (0x3F = 1.875 in fp8-e4m3, 0.0 stays 0) -- a lossless 4x shrink of
    the dominant HBM stream (128 MB -> 32 MB per core).  The host also
    transposes each core's slice to [V, 2048] so the vocab dim lands on
    SBUF partitions.
  - TensorE scans x: per 128-wide v-chunk, one matmul against a
    sliding-window fp8 stationary whose 3 live columns are
    (ones, j>>4, j&15) -- all e4m3-exact -- accumulates the chunk's
    (presence, m, r) digit rows into a PSUM bank; 32 chunks share a
    bank via zero columns.  128 chunks x 4 token-groups = 512 matmuls.
  - Decode: per fill, ACT drains the bank to SBUF and a tiny f32 matmul
    (table as stationary, decode weights as rhs) accumulates
    idx = 128*chunk + 16*m + r into a per-token [128,1] PSUM column.
    A +0.25 bias absorbs the 1/1.875 weight rounding before the int32
    cast.
  - indirect-DMA gathers the matching bf16 rows of W^T [V, E]; one DVE
    add applies the host-preadded pos+fmap table; DMA out as f32.
"""

import numpy as np
import ml_dtypes

B, S, F, V, E = 4, 64, 64, 16384, 512
NCORES = 8
TOKENS = B * S * F            # 16384
TPC = TOKENS // NCORES        # 2048 tokens per core
P = 128                       # partitions
NCH = V // P                  # 128 v-chunks
NFILL = 4                     # psum fills per token-group sweep
CPF = NCH // NFILL            # 32 chunks per fill
NG = 4                        # token groups of 512
GTOK = TPC // NG              # 512
NSUB = 4                      # 128-token subgroups per group
DMA_B = 4                     # v-chunks per x DMA (1 MB transfers)

FP8_ONE = 1.875               # value of byte 0x3F as e4m3

_cache = {}


def _build():
    import concourse.bass as bass
    import concourse.bacc as bacc
    import concourse.mybir as mybir
    import concourse.tile as tile
    from concourse.alu_op_type import AluOpType

    fp8 = mybir.dt.float8e4
    bf16 = mybir.dt.bfloat16
    f32 = mybir.dt.float32

    nc = bacc.Bacc(trn_type="TRN2")
    xT = nc.declare_dram_parameter("xT", [V, TPC], fp8, isOutput=False)
    wt = nc.declare_dram_parameter("wt", [V, E], bf16, isOutput=False)
    combo = nc.declare_dram_parameter("combo", [TPC, E], bf16, isOutput=False)
    stat = nc.declare_dram_parameter("stat", [P, 252], fp8, isOutput=False)
    wdec = nc.declare_dram_parameter("wdec", [P, NFILL], f32, isOutput=False)
    out = nc.declare_dram_parameter("out", [TPC, E], f32, isOutput=True)

    # views
    xT_r = xT.rearrange("(nb four p) t -> nb p four t", four=DMA_B, p=P)
    comb_r = combo.rearrange("(gs p) e -> p gs e", p=P)          # [128,16,512]
    out_r = out.rearrange("(g s p) e -> g p s e", g=NG, s=NSUB, p=P)
    wt_flat = wt[:, :]

    rings = [nc.sync, nc.scalar]

    with tile.TileContext(nc) as tc:
        with (
            tc.tile_pool(name="pconst", bufs=1) as pconst,
            tc.tile_pool(name="px", bufs=6) as px,
            tc.tile_pool(name="ptab", bufs=2) as ptab,
            tc.tile_pool(name="pio", bufs=2) as pio,
            tc.tile_pool(name="pscan", bufs=1, space="PSUM") as pscan,
            tc.tile_pool(name="pidx", bufs=1, space="PSUM") as pidx,
        ):
            stat_sb = pconst.tile([P, 252], fp8)
            nc.gpsimd.dma_start(out=stat_sb[:, :], in_=stat[:, :])
            wdec_sb = pconst.tile([P, NFILL], f32)
            nc.gpsimd.dma_start(out=wdec_sb[:, :], in_=wdec[:, :])
            comb_sb = pconst.tile([P, NG * NSUB, E], bf16)
            nc.gpsimd.dma_start(out=comb_sb[:, :, :], in_=comb_r[:, :, :])
            zbuf = pconst.tile([P, P], f32)
            nc.vector.memset(zbuf[:, :], 0.0)
            idx_sb = pconst.tile([P, NG * NSUB], mybir.dt.int32)

            # per-token-group idx accumulators; zero-matmul sets has_written
            # on the whole bank so later decode matmuls accumulate per column
            idxps = [
                pidx.tile([P, NSUB], f32, tag=f"idx{g}", name=f"idx{g}")
                for g in range(NG)
            ]
            for g in range(NG):
                nc.tensor.matmul(
                    idxps[g][:, :], lhsT=zbuf[:, :], rhs=zbuf[:, 0:NSUB],
                    start=True, stop=False, skip_group_check=True,
                )

            for f in range(NFILL):
                banks = [
                    pscan.tile([P, GTOK], f32, tag=f"scan{g}", name=f"scan{g}")
                    for g in range(NG)
                ]
                for dt_ in range(CPF // DMA_B):
                    xt = px.tile([P, DMA_B, TPC], fp8, tag="x")
                    rings[dt_ % 2].dma_start(
                        out=xt[:, :, :], in_=xT_r[f * (CPF // DMA_B) + dt_]
                    )
                    for j in range(DMA_B):
                        cp = dt_ * DMA_B + j          # chunk-in-fill 0..31
                        lhsT = stat_sb[:, 124 - 4 * cp:252 - 4 * cp]
                        for g in range(NG):
                            nc.tensor.matmul(
                                banks[g][:, :],
                                lhsT=lhsT,
                                rhs=xt[:, j, g * GTOK:(g + 1) * GTOK],
                                start=(cp == 0),
                                stop=(cp == CPF - 1),
                                skip_group_check=True,
                            )
                for g in range(NG):
                    tab = ptab.tile([P, GTOK], f32, tag=f"tab{g}")
                    nc.scalar.copy(out=tab[:, :], in_=banks[g][:, :])
                    for s in range(NSUB):
                        nc.tensor.matmul(
                            idxps[g][:, s:s + 1],
                            lhsT=tab[:, s * P:(s + 1) * P],
                            rhs=wdec_sb[:, f:f + 1],
                            start=False,
                            stop=(f == NFILL - 1 and s == NSUB - 1),
                            skip_group_check=True,
                        )

            for g in range(NG):
                for s in range(NSUB):
                    nc.vector.tensor_scalar(
                        out=idx_sb[:, 4 * g + s:4 * g + s + 1],
                        in0=idxps[g][:, s:s + 1],
                        scalar1=0.25, scalar2=None, op0=AluOpType.add,
                    )
                gath = pio.tile([P, NSUB, E], bf16, tag="gath")
                for s in range(NSUB):
                    nc.gpsimd.indirect_dma_start(
                        out=gath[:, s, :],
                        out_offset=None,
                        in_=wt_flat,
                        in_offset=bass.IndirectOffsetOnAxis(
                            ap=idx_sb[:, 4 * g + s:4 * g + s + 1], axis=0
                        ),
                    )
                outt = pio.tile([P, NSUB, E], f32, tag="out")
                for s in range(NSUB):
                    nc.vector.tensor_tensor(
                        out=outt[:, s, :],
                        in0=gath[:, s, :],
                        in1=comb_sb[:, 4 * g + s, :],
                        op=AluOpType.add,
                    )
                rings[g % 2].dma_start(out=out_r[g], in_=outt[:, :, :])
    nc.finalize()
    return nc


def _prep_xt(x_flat):
    """[TOKENS, V] f32 one-hot -> per-core [V, TPC] fp8-e4m3 byte views."""
    try:
        import jax
        import jax.numpy as jnp
        cpu = jax.devices("cpu")[0]

        def _f(xc):
            u = jax.lax.bitcast_convert_type(xc, jnp.uint32)
            return (u >> 24).astype(jnp.uint8).T

        jf = jax.jit(_f)
        outs = []
        with jax.default_device(cpu):
            for c in range(NCORES):
                a = np.asarray(jf(x_flat[c * TPC:(c + 1) * TPC]))
                outs.append(a.view(ml_dtypes.float8_e4m3))
        return outs
    except Exception:
        xb = x_flat.view(np.uint8).reshape(TOKENS, V, 4)[:, :, 3]
        outs = []
        for c in range(NCORES):
            a = np.ascontiguousarray(xb[c * TPC:(c + 1) * TPC])
            aT = np.empty((V, TPC), np.uint8)
            for i0 in range(0, TPC, 128):
                aT[:, i0:i0 + 128] = a[i0:i0 + 128, :].T
            outs.append(aT.view(ml_dtypes.float8_e4m3))
        return outs


def _host_shards(x, W, pos_emb, fmap_emb):
    x_flat = np.ascontiguousarray(x.reshape(TOKENS, V))
    xts = _prep_xt(x_flat)
    wt = np.ascontiguousarray(W.T).astype(ml_dtypes.bfloat16)    # [V, E]
    fmap_t = np.tile(fmap_emb[:F], (TPC // F, 1))                # [2048, E]

    # stationary: rows j, cols 124..126 = (1, j>>4, j&15); window slide
    # stat[:, 124-4c : 252-4c] puts them at output partitions 4c+0..2
    st = np.zeros((P, 252), np.float32)
    st[:, 124] = 1.0
    st[:, 125] = np.arange(P) >> 4
    st[:, 126] = np.arange(P) & 15
    stat = st.astype(ml_dtypes.float8_e4m3)

    # decode weights: psum row 4c+d of fill f -> contribution to idx
    rows = np.arange(P)
    cpr, d = rows // 4, rows % 4
    wdec = np.zeros((P, NFILL), np.float32)
    for f in range(NFILL):
        col = np.select(
            [d == 0, d == 1, d == 2],
            [128.0 * (CPF * f + cpr), 16.0, 1.0], 0.0,
        )
        wdec[:, f] = (col / FP8_ONE).astype(np.float32)

    in_maps = []
    for c in range(NCORES):
        s_base = (c % 2) * 32
        pos2 = np.repeat(pos_emb[s_base:s_base + TPC // F], F, axis=0)
        combo = (pos2 + fmap_t).astype(ml_dtypes.bfloat16)
        in_maps.append({
            "xT": xts[c],
            "wt": wt,
            "combo": np.ascontiguousarray(combo),
            "stat": stat,
            "wdec": wdec,
        })
    return in_maps


def kernel(x, W, pos_emb, fmap_emb):
    from concourse import bass_utils

    x = np.asarray(x, dtype=np.float32)
    W = np.asarray(W, dtype=np.float32)
    pos_emb = np.asarray(pos_emb, dtype=np.float32)
    fmap_emb = np.asarray(fmap_emb, dtype=np.float32)

    if "nc" not in _cache:
        _cache["nc"] = _build()
    nc = _cache["nc"]

    in_maps = _host_shards(x, W, pos_emb, fmap_emb)
    res = bass_utils.run_bass_kernel_spmd(nc, in_maps, core_ids=list(range(NCORES)))
    outs = [res.results[c]["out"] for c in range(NCORES)]
    full = np.concatenate(outs, axis=0).reshape(B, S, F, E)
    return full
